# revision 23
# baseline (speedup 1.0000x reference)
"""Bass/Trainium2 kernel for nn_BiencoderRanker: pairwise cosine similarity.

scores[n, m] = <pred_n, cand_m> / (|pred_n| * |cand_m|)
  fp_pred: (1024, 4096) fp32, fp_cand: (16384, 4096) fp32 -> scores (1024, 16384) fp32

Sharding: fp_cand split along M across 8 cores (2048 rows each); fp_pred
replicated. Each core computes its (1024, 2048) tile; host concatenates.

v7 (fp8 DoubleRow GEMM, single-read cand path). On this part the binding
resources are the single shared DMA device and per-instruction engine taxes,
so: cand norms are computed from the ALREADY-LOADED transposed bT tiles
(squares on ACT/DVE + ones-stationary DoubleRow matmuls that partition-reduce
into one PSUM bank, emerging pre-broadcast — no natural-layout cand read, no
DRAM roundtrip). Pred keeps the tiny natural-layout path (4.2MB) since its
inverse must land on the partition axis. Output path stays minimum-op-count:
[128,1024] psum units x3 bufs, plain wide ACT Copy drains, one fused DVE
scalar_tensor_tensor join per unit, paired stores.
"""

import numpy as np
import ml_dtypes

import concourse.bacc as bacc
import concourse.mybir as mybir
import concourse.tile as tile
from concourse.bass_utils import run_bass_kernel_spmd

P = 128
N = 1024  # fp_pred rows
K = 4096  # feature dim
M_FULL = 16384  # fp_cand rows
N_CORES = 8
M = M_FULL // N_CORES  # cand rows per core
NB = N // P  # 8 pred row-chunks
MBLK = 4  # m-blocks per core (512 cand rows each)
KC = K // P  # 32 contraction chunks of 128
KP = KC // 2  # 16 DoubleRow chunks of 256
FREE = 512
WIDE = 1024  # half the per-core M

F32 = mybir.dt.float32
BF16 = mybir.dt.bfloat16
F8 = mybir.dt.float8e4
AF = mybir.ActivationFunctionType
NP_F8 = ml_dtypes.float8_e4m3

_compiled = None


def _build(repeats=1):
    nc = bacc.Bacc(None, target_bir_lowering=False)
    pred_t = nc.dram_tensor("pred_t", (P, NB, KC, P), F8, kind="ExternalInput")
    cand_t = nc.dram_tensor("cand_t", (P, MBLK, KC, FREE), F8, kind="ExternalInput")
    pred_nat = nc.dram_tensor("pred_nat", (N, K), F8, kind="ExternalInput")
    out = nc.dram_tensor("scores", (N, M), BF16, kind="ExternalOutput")

    with tile.TileContext(nc) as tc:
        with (
            tc.tile_pool(name="at", bufs=1) as at_pool,
            tc.tile_pool(name="bt", bufs=3) as bt_pool,
            tc.tile_pool(name="stage", bufs=2) as stage_pool,
            tc.tile_pool(name="sq", bufs=2) as sq_pool,
            tc.tile_pool(name="bsq", bufs=3) as bsq_pool,
            tc.tile_pool(name="norm", bufs=2) as norm_pool,
            tc.tile_pool(name="invs", bufs=2) as inv_pool,
            tc.tile_pool(name="outb", bufs=6) as outb_pool,
            tc.tile_pool(name="psum", bufs=3, space="PSUM") as psum_pool,
            tc.tile_pool(name="psn", bufs=2, space="PSUM") as psn_pool,
        ):
            aT = at_pool.tile([P, NB, KC, P], F8, name="aT", bufs=1)
            ssq_a = norm_pool.tile([P, NB], F32, name="ssq_a", bufs=1)
            inv_a = norm_pool.tile([P, NB], F32, name="inv_a", bufs=1)
            ones = norm_pool.tile([P, 2, P], F8, name="ones", bufs=1)
            nc.vector.memset(ones[:], 1.0)
            bts = {}

            def bt_tile(mb):
                if mb not in bts:
                    bts[mb] = bt_pool.tile([P, KC, FREE], F8, tag="bt", name=f"bT{mb}")
                return bts[mb]

            def t_a(nb):
                nc.sync.dma_start(aT[:, nb], pred_t[:, nb])

            def t_b(mb, split=1):
                bT = bt_tile(mb)
                step = KC // split
                for s in range(split):
                    nc.sync.dma_start(
                        bT[:, s * step : (s + 1) * step, :],
                        cand_t[:, mb, s * step : (s + 1) * step, :],
                    )

            # ---- pred norms: natural-layout path (partition-axis result) ----
            def norm_a_pair(nbp):
                nat2 = stage_pool.tile([P, 2, K], F8, tag="stage", name=f"nat{nbp}")
                nc.scalar.dma_start(
                    nat2[:],
                    pred_nat[nbp * 2 * P : (nbp + 1) * 2 * P, :].rearrange(
                        "(h p) k -> p h k", p=P
                    ),
                )
                for h in range(2):
                    dst = ssq_a[:, 2 * nbp + h : 2 * nbp + h + 1]
                    if h == 0:
                        sq = sq_pool.tile(
                            [P, K], F8, tag="sqa", name=f"sqa{nbp}", bufs=1
                        )
                        nc.scalar.activation(
                            sq[:], nat2[:, 0, :], AF.Square, accum_out=dst
                        )
                    else:
                        sq = sq_pool.tile(
                            [P, K], F8, tag="sqd", name=f"sqd{nbp}", bufs=1
                        )
                        nc.vector.scalar_tensor_tensor(
                            sq[:],
                            nat2[:, 1, :],
                            1.0,
                            nat2[:, 1, :],
                            mybir.AluOpType.mult,
                            mybir.AluOpType.mult,
                            accum_out=dst,
                        )

            # ---- cand norms: from transposed tiles, no extra HBM reads ----
            def inv_b_wide(mh):
                """[P, WIDE] f32 tile of 1/|cand_m| rows (already broadcast)."""
                invbc = inv_pool.tile([P, WIDE], F32, tag="invbc", name=f"invbc{mh}")
                for hmb in range(2):
                    mb = 2 * mh + hmb
                    bT = bts[mb]
                    psn = psn_pool.tile(
                        [P, FREE], F32, tag="psn", name=f"psn{mb}", bufs=2
                    )
                    for half in range(2):
                        bsq = bsq_pool.tile(
                            [P, KP, FREE], F8, tag="bsq", name=f"bsq{mb}_{half}"
                        )
                        src = bT[:, 16 * half : 16 * (half + 1), :]
                        if half == 0:
                            nc.scalar.activation(bsq[:], src, AF.Square)
                        else:
                            nc.vector.scalar_tensor_tensor(
                                bsq[:],
                                src,
                                1.0,
                                src,
                                mybir.AluOpType.mult,
                                mybir.AluOpType.mult,
                            )
                        for u in range(KP // 2):
                            t = half * (KP // 2) + u
                            nc.tensor.matmul(
                                psn[:],
                                ones[:],
                                bsq[:, 2 * u : 2 * u + 2, :],
                                start=(t == 0),
                                stop=(t == KP - 1),
                                perf_mode=mybir.MatmulPerfMode.DoubleRow,
                            )
                    nrm = inv_pool.tile(
                        [P, FREE], F32, tag="nrmb", name=f"nrmb{mb}", bufs=2
                    )
                    nc.scalar.activation(nrm[:], psn[:], AF.Sqrt)
                    nc.vector.reciprocal(
                        invbc[:, hmb * FREE : (hmb + 1) * FREE], nrm[:]
                    )
                return invbc

            # ---- matmul path: [128, 1024] psum units, paired stores ----
            def mm_pair(nbp, mh, inv_bcast):
                ob = outb_pool.tile([P, 2, WIDE], BF16, tag="otb", name=f"ob{mh}_{nbp}")
                for j in range(2):
                    nb = 2 * nbp + j
                    ps = psum_pool.tile(
                        [P, WIDE], F32, tag="ps", name=f"ps{mh}_{nb}", bufs=3
                    )
                    for h in range(2):
                        bT = bts[2 * mh + h]
                        for kp in range(KP):
                            nc.tensor.matmul(
                                ps[:, h * FREE : (h + 1) * FREE],
                                aT[:, nb, 2 * kp : 2 * kp + 2, :],
                                bT[:, 2 * kp : 2 * kp + 2, :],
                                start=(kp == 0),
                                stop=(kp == KP - 1),
                                perf_mode=mybir.MatmulPerfMode.DoubleRow,
                            )
                    nc.scalar.activation(ob[:, j, :], ps[:], AF.Copy)
                    nc.vector.scalar_tensor_tensor(
                        ob[:, j, :],
                        ob[:, j, :],
                        inv_a[:, nb : nb + 1],
                        inv_bcast[:],
                        mybir.AluOpType.mult,
                        mybir.AluOpType.mult,
                    )
                nc.scalar.dma_start(
                    out[
                        2 * nbp * P : 2 * (nbp + 1) * P, mh * WIDE : (mh + 1) * WIDE
                    ].rearrange("(j p) m -> p j m", p=P),
                    ob[:],
                )

            # ---- emission ----
            for _rep in range(repeats):
                bts.clear()

                t_a(0)
                t_b(0, split=2)
                t_b(1)
                for nb in range(1, NB):
                    t_a(nb)
                for nbp in range(NB // 2):
                    norm_a_pair(nbp)
                nrm_a = norm_pool.tile([P, NB], F32, tag="nrma", name="nrm_a", bufs=1)
                nc.scalar.activation(nrm_a[:], ssq_a[:], AF.Sqrt)
                nc.vector.reciprocal(inv_a[:], nrm_a[:])

                for mh in range(2):
                    if mh == 0:
                        t_b(2)
                        t_b(3)
                    bc = inv_b_wide(mh)
                    for nbp in range(NB // 2):
                        mm_pair(nbp, mh, bc)
    nc.compile()
    return nc


def _get_compiled():
    global _compiled
    if _compiled is None:
        _compiled = _build()
    return _compiled


def _in_maps(fp_pred: np.ndarray, fp_cand: np.ndarray) -> list[dict]:
    """Host marshalling: fp8 cast + K-major tiled copies, per core."""
    pred_f8 = np.asarray(fp_pred, dtype=np.float32).astype(NP_F8)
    cand_f8 = np.asarray(fp_cand, dtype=np.float32).astype(NP_F8)
    # pred_t[p, nb, kc, q] = pred[nb*128+q, kc*128+p]
    pred_t = np.ascontiguousarray(
        pred_f8.reshape(NB, P, KC, P).transpose(3, 0, 2, 1)
    )
    maps = []
    for i in range(N_CORES):
        cshard = np.ascontiguousarray(cand_f8[i * M : (i + 1) * M])
        cand_t = np.ascontiguousarray(
            cshard.reshape(MBLK, FREE, KC, P).transpose(3, 0, 2, 1)
        )
        maps.append(
            {
                "pred_t": pred_t,
                "cand_t": cand_t,
                "pred_nat": pred_f8,
            }
        )
    return maps


def kernel(fp_pred: np.ndarray, fp_cand: np.ndarray) -> np.ndarray:
    fp_pred = np.asarray(fp_pred, dtype=np.float32)
    fp_cand = np.asarray(fp_cand, dtype=np.float32)
    assert fp_pred.shape == (N, K) and fp_cand.shape == (M_FULL, K)

    nc = _get_compiled()
    res = run_bass_kernel_spmd(nc, _in_maps(fp_pred, fp_cand), core_ids=list(range(N_CORES)))
    return np.concatenate(
        [res.results[i]["scores"].astype(np.float32) for i in range(N_CORES)], axis=1
    )


# revision 24
# speedup vs baseline: 1.2004x; 1.2004x over previous
"""Bass/Trainium2 kernel for nn_BiencoderRanker: pairwise cosine similarity.

scores[n, m] = <pred_n, cand_m> / (|pred_n| * |cand_m|)
  fp_pred: (1024, 4096) fp32, fp_cand: (16384, 4096) fp32 -> scores (1024, 16384) fp32

Sharding: fp_cand split along M across 8 cores (2048 rows each); fp_pred
replicated. Each core computes its (1024, 2048) tile; host concatenates.

v6 (fp8 DoubleRow GEMM, minimum-op-count output path). HW findings driving
this shape: the GEMM core runs at the PE roofline (~55us/core) but every
non-PE instruction and every DMA carries ~0.8-2.5us of unmodeled overhead
(Pool ~3x worse than DVE/ACT), so throughput comes from FEW, WIDE ops:
- psum units [128, 2(nb), 1024(m)] = 4 banks x 2 bufs: 8 plain ACT Copy
  drains and 8 paired stores for the whole output.
- join = one DVE scalar_tensor_tensor per [128,1024] slice (16 total):
  ob = (raw * 1/|pred_n|) * bcast(1/|cand_m|), in place, all bf16.
- norms: 12 paired natural-layout fp8 loads; one fused square+rowsum op per
  128-row chunk, split ACT (Square+accum) / DVE (scalar_tensor_tensor+accum),
  no Pool tensor work (gpsimd only issues DMAs + partition_broadcast).
- batched sqrt/reciprocal; 1/|cand| DRAM roundtrip reloaded as bf16 row.
"""

import numpy as np
import ml_dtypes

import concourse.bacc as bacc
import concourse.mybir as mybir
import concourse.tile as tile
from concourse.bass_utils import run_bass_kernel_spmd
from concourse.tile_rust import add_dep_helper

P = 128
N = 1024  # fp_pred rows
K = 4096  # feature dim
M_FULL = 16384  # fp_cand rows
N_CORES = 8
M = M_FULL // N_CORES  # cand rows per core
NB = N // P  # 8 pred row-chunks
MBLK = 4  # m-blocks per core (512 cand rows each)
MC = 4  # 128-row chunks per m-block
KC = K // P  # 32 contraction chunks of 128
KP = KC // 2  # 16 DoubleRow chunks of 256
FREE = 512
WIDE = 1024  # half the per-core M

F32 = mybir.dt.float32
BF16 = mybir.dt.bfloat16
F8 = mybir.dt.float8e4
AF = mybir.ActivationFunctionType
NP_F8 = ml_dtypes.float8_e4m3

_compiled = None


def _build(repeats=1):
    nc = bacc.Bacc(None, target_bir_lowering=False)
    pred_t = nc.dram_tensor("pred_t", (P, NB, KC, P), F8, kind="ExternalInput")
    cand_t = nc.dram_tensor("cand_t", (P, MBLK, KC, FREE), F8, kind="ExternalInput")
    pred_nat = nc.dram_tensor("pred_nat", (N, K), F8, kind="ExternalInput")
    cand_nat = nc.dram_tensor("cand_nat", (M, K), F8, kind="ExternalInput")
    out = nc.dram_tensor("scores", (N, M), BF16, kind="ExternalOutput")

    with tile.TileContext(nc) as tc:
        with (
            tc.tile_pool(name="dram", bufs=1, space="DRAM") as dram_pool,
            tc.tile_pool(name="at", bufs=1) as at_pool,
            tc.tile_pool(name="bt", bufs=3) as bt_pool,
            tc.tile_pool(name="stage", bufs=3) as stage_pool,
            tc.tile_pool(name="sq", bufs=2) as sq_pool,
            tc.tile_pool(name="norm", bufs=2) as norm_pool,
            tc.tile_pool(name="invs", bufs=2) as inv_pool,
            tc.tile_pool(name="outb", bufs=4) as outb_pool,
            tc.tile_pool(name="psum", bufs=2, space="PSUM") as psum_pool,
        ):
            invb_dram = dram_pool.tile([M], F32, name="invb_rt")
            aT = at_pool.tile([P, NB, KC, P], F8, name="aT", bufs=1)
            ssq_a = norm_pool.tile([P, NB], F32, name="ssq_a", bufs=1)
            inv_a = norm_pool.tile([P, NB], F32, name="inv_a", bufs=1)
            bts = {}

            def bt_tile(mb):
                if mb not in bts:
                    bts[mb] = bt_pool.tile([P, KC, FREE], F8, tag="bt", name=f"bT{mb}")
                return bts[mb]

            def t_a(nb):
                nc.sync.dma_start(aT[:, nb], pred_t[:, nb])

            def t_b(mb, split=1):
                bT = bt_tile(mb)
                step = KC // split
                for s in range(split):
                    nc.sync.dma_start(
                        bT[:, s * step : (s + 1) * step, :],
                        cand_t[:, mb, s * step : (s + 1) * step, :],
                    )

            # ---- norm path: paired 256-row loads, one fused op per chunk ----
            def norm_pair(dram_rows2, ssq_dsts, idx, engs, queue=None):
                q = queue or nc.gpsimd
                nat2 = stage_pool.tile([P, 2, K], F8, tag="stage", name=f"nat{idx}")
                q.dma_start(nat2[:], dram_rows2)
                for h in range(2):
                    nat = nat2[:, h, :]
                    if engs[h] == "a":
                        sq = sq_pool.tile(
                            [P, K], F8, tag="sqa", name=f"sqa{idx}_{h}", bufs=1
                        )
                        nc.scalar.activation(
                            sq[:], nat, AF.Square, accum_out=ssq_dsts[h]
                        )
                    else:
                        sq = sq_pool.tile(
                            [P, K], F8, tag="sqd", name=f"sqd{idx}_{h}", bufs=1
                        )
                        nc.vector.scalar_tensor_tensor(
                            sq[:],
                            nat,
                            1.0,
                            nat,
                            mybir.AluOpType.mult,
                            mybir.AluOpType.mult,
                            accum_out=ssq_dsts[h],
                        )

            def norm_a_pair(nbp):
                norm_pair(
                    pred_nat[nbp * 2 * P : (nbp + 1) * 2 * P, :].rearrange(
                        "(h p) k -> p h k", p=P
                    ),
                    [ssq_a[:, 2 * nbp + h : 2 * nbp + h + 1] for h in range(2)],
                    f"a{nbp}",
                    ["a", "d"],
                    queue=nc.scalar,  # keep the SWDGE queue free for cand loads
                )

            ssq_bs = {}

            def norm_b_pair(mb, mcp):
                if mb not in ssq_bs:
                    ssq_bs[mb] = norm_pool.tile(
                        [P, MC], F32, tag="ssqb", name=f"ssqb{mb}", bufs=2
                    )
                r0 = (mb * MC + 2 * mcp) * P
                norm_pair(
                    cand_nat[r0 : r0 + 2 * P, :].rearrange("(h p) k -> p h k", p=P),
                    [ssq_bs[mb][:, 2 * mcp + h : 2 * mcp + h + 1] for h in range(2)],
                    f"b{mb}_{mcp}",
                    ["a", "d"] if (mb + mcp) % 2 == 0 else ["d", "a"],
                )

            def inv_b_finish(mb):
                nrm = norm_pool.tile([P, MC], F32, tag="nrmb", name=f"nrmb{mb}", bufs=2)
                nc.scalar.activation(nrm[:], ssq_bs[mb][:], AF.Sqrt)
                invb = inv_pool.tile([P, MC], F32, tag="invb", name=f"invb{mb}", bufs=2)
                nc.vector.reciprocal(invb[:], nrm[:])
                nc.scalar.dma_start(
                    invb_dram[mb * FREE : (mb + 1) * FREE].rearrange(
                        "(mc p) -> p mc", p=P
                    ),
                    invb[:],
                )

            def invbc_wide(mh):
                row = inv_pool.tile([1, WIDE], BF16, tag="invrow", name=f"invrow{mh}")
                nc.gpsimd.dma_start(
                    row[:], invb_dram[None, mh * WIDE : (mh + 1) * WIDE]
                )
                bcast = inv_pool.tile([P, WIDE], BF16, tag="invbc", name=f"invbc{mh}")
                nc.gpsimd.partition_broadcast(bcast[:], row[:])
                return bcast

            # ---- matmul path: [128, 2(nb), 1024(m)] 4-bank psum units ----
            def mm_unit(nbp, mh, inv_bcast):
                ps = psum_pool.tile(
                    [P, 2, WIDE], F32, tag="ps", name=f"ps{mh}_{nbp}", bufs=2
                )
                for j in range(2):
                    nb = 2 * nbp + j
                    for h in range(2):
                        bT = bts[2 * mh + h]
                        for kp in range(KP):
                            nc.tensor.matmul(
                                ps[:, j, h * FREE : (h + 1) * FREE],
                                aT[:, nb, 2 * kp : 2 * kp + 2, :],
                                bT[:, 2 * kp : 2 * kp + 2, :],
                                start=(kp == 0),
                                stop=(kp == KP - 1),
                                perf_mode=mybir.MatmulPerfMode.DoubleRow,
                            )
                ob = outb_pool.tile([P, 2, WIDE], BF16, tag="otb", name=f"ob{mh}_{nbp}")
                nc.scalar.activation(ob[:], ps[:], AF.Copy)  # one wide raw drain
                for j in range(2):
                    nb = 2 * nbp + j
                    nc.vector.scalar_tensor_tensor(
                        ob[:, j, :],
                        ob[:, j, :],
                        inv_a[:, nb : nb + 1],
                        inv_bcast[:],
                        mybir.AluOpType.mult,
                        mybir.AluOpType.mult,
                    )
                nc.scalar.dma_start(
                    out[
                        2 * nbp * P : 2 * (nbp + 1) * P, mh * WIDE : (mh + 1) * WIDE
                    ].rearrange("(j p) m -> p j m", p=P),
                    ob[:],
                )

            # ---- emission ----
            for _rep in range(repeats):
                bts.clear()
                ssq_bs.clear()

                t_a(0)
                t_b(0, split=2)
                t_b(1)
                for nb in range(1, NB):
                    t_a(nb)
                for nbp in range(NB // 2):
                    norm_a_pair(nbp)
                nrm_a = norm_pool.tile([P, NB], F32, tag="nrma", name="nrm_a", bufs=1)
                nc.scalar.activation(nrm_a[:], ssq_a[:], AF.Sqrt)
                nc.vector.reciprocal(inv_a[:], nrm_a[:])

                for mh in range(2):
                    if mh == 0:
                        t_b(2)
                        t_b(3)
                    for h in range(2):
                        mb = 2 * mh + h
                        for mcp in range(MC // 2):
                            norm_b_pair(mb, mcp)
                        inv_b_finish(mb)
                    bc = invbc_wide(mh)
                    for nbp in range(NB // 2):
                        mm_unit(nbp, mh, bc)
    nc.compile()
    return nc


def _get_compiled():
    global _compiled
    if _compiled is None:
        _compiled = _build()
    return _compiled


def _in_maps(fp_pred: np.ndarray, fp_cand: np.ndarray) -> list[dict]:
    """Host marshalling: fp8 cast + K-major tiled copies, per core."""
    pred_f8 = np.asarray(fp_pred, dtype=np.float32).astype(NP_F8)
    cand_f8 = np.asarray(fp_cand, dtype=np.float32).astype(NP_F8)
    # pred_t[p, nb, kc, q] = pred[nb*128+q, kc*128+p]
    pred_t = np.ascontiguousarray(
        pred_f8.reshape(NB, P, KC, P).transpose(3, 0, 2, 1)
    )
    maps = []
    for i in range(N_CORES):
        cshard = np.ascontiguousarray(cand_f8[i * M : (i + 1) * M])
        cand_t = np.ascontiguousarray(
            cshard.reshape(MBLK, FREE, KC, P).transpose(3, 0, 2, 1)
        )
        maps.append(
            {
                "pred_t": pred_t,
                "cand_t": cand_t,
                "pred_nat": pred_f8,
                "cand_nat": cshard,
            }
        )
    return maps


def kernel(fp_pred: np.ndarray, fp_cand: np.ndarray) -> np.ndarray:
    fp_pred = np.asarray(fp_pred, dtype=np.float32)
    fp_cand = np.asarray(fp_cand, dtype=np.float32)
    assert fp_pred.shape == (N, K) and fp_cand.shape == (M_FULL, K)

    nc = _get_compiled()
    res = run_bass_kernel_spmd(nc, _in_maps(fp_pred, fp_cand), core_ids=list(range(N_CORES)))
    return np.concatenate(
        [res.results[i]["scores"].astype(np.float32) for i in range(N_CORES)], axis=1
    )
